# revision 1
# baseline (speedup 1.0000x reference)
# Mistral sliding-window attention (B=1, S=2048, H=4096, 32 q heads / 8 kv
# heads, window 4096 -> plain causal at this S) on 8 Trainium2 NeuronCores.
#
# Sharding: tensor-parallel over heads. Core c owns q heads 4c..4c+3 and kv
# head c. hidden_states is replicated (transposed on host to [H, S] so the
# contraction dim is the partition dim). Each core computes its attention
# output slice attn.T [512, S]; per-head AllGathers assemble the full
# [4096, S] while later heads still compute, and each core accumulates a
# 512-column slice of o_proj head-by-head; the host concatenates the 8
# column slices into the full output.
#
# All big matmuls run as float32r (fp32 storage, full-rate PE) with the
# moving dim = 512. Scores are computed transposed (S.T[kv, q]) so that the
# P@V contraction needs no transposes of the probability tiles; softmax
# denominators come from an all-ones stationary matmul accumulated alongside
# P@V, and the causal mask is a host-precomputed staircase slice multiplied
# in after exp. Attention runs two-pass per (head, q-chunk) — all score
# matmuls + exps first, then the PV/sum matmuls — so the PE never stalls on
# the ACT engine mid-chain.

from contextlib import ExitStack

import numpy as np

import concourse.bacc as bacc
import concourse.bass as bass
import concourse.mybir as mybir
import concourse.tile as tile
from concourse.bass_utils import run_bass_kernel_spmd
from concourse.masks import make_identity

HIDDEN = 4096
NH = 32
NKV = 8
HD = 128
THETA = 10000.0
S = 2048
NCORES = 8

QH = NH // NCORES          # 4 q heads per core
DQ = QH * HD               # 512 (per-core q/attn width)
DOUT = DQ + 2 * HD         # 768 = q heads + k + v projection width
MT = DOUT // 128           # 6 projection m-tiles (0..3 q, 4 k, 5 v)
KT = HIDDEN // 128         # 32 contraction tiles
KG = 4                     # x-load group: k-tiles per DMA
TCH = 512                  # token chunk (matmul moving dim)
NTCH = S // TCH            # 4
KVT = S // 128             # 16 kv tiles
SCALE = 1.0 / float(np.sqrt(HD))

F32 = mybir.dt.float32
F32R = mybir.dt.float32r
EXP = mybir.ActivationFunctionType.Exp


def _rope(nc, pool, src, dst, cs, sn):
    """dst = src*cos + rotate_half(src)*sin, in [d, tok] layout.

    src/dst are [128, n]; cs/sn are [64, n] (the two 64-row halves share
    frequencies). rotate_half: rows 0:64 get -src[64:128], rows 64:128 get
    src[0:64].
    """
    top, bot = src[0:64, :], src[64:128, :]
    ta = pool.tile([64, TCH], F32, name="rope_a")
    tb = pool.tile([64, TCH], F32, name="rope_b")
    nc.vector.tensor_mul(ta, top, cs)
    nc.vector.tensor_mul(tb, bot, sn)
    nc.vector.tensor_sub(dst[0:64, :], ta, tb)
    nc.vector.tensor_mul(ta, bot, cs)
    nc.vector.tensor_mul(tb, top, sn)
    nc.vector.tensor_add(dst[64:128, :], ta, tb)


def build_kernel_body(ctx: ExitStack, tc: tile.TileContext, outs, ins):
    nc = tc.nc
    xT, wqkv, ow, cos_t, sin_t, stair = (
        ins["xT"], ins["wqkv"], ins["ow"], ins["cos_t"], ins["sin_t"], ins["stair"],
    )
    out = outs["out"]

    # per-head bounce + gather buffers so each head's AllGather can fire as
    # soon as that head's attention is done (overlaps comm with compute)
    attn_loc = [nc.dram_tensor(f"attn_loc{h}", [HD, S], F32).ap()
                for h in range(QH)]
    attn_gat = [nc.dram_tensor(f"attn_gat{h}", [NCORES * HD, S], F32,
                               addr_space="Shared").ap()
                for h in range(QH)]

    singles = ctx.enter_context(tc.tile_pool(name="singles", bufs=1))
    stair_sb = singles.tile([128, 896], F32)
    nc.sync.dma_start(out=stair_sb, in_=stair)
    ones_sb = singles.tile([128, 128], F32R)

    # persistent projection outputs, [d, tok] layout
    qT = singles.tile([128, QH, S], F32R)    # q head h -> qT[:, h, :]
    kT = singles.tile([128, S], F32R)
    V = singles.tile([128, KVT, HD], F32R)   # V[:, j, :] = [tok 128, d 128]

    # ---- phase 1: QKV projection + RoPE --------------------------------
    with (
        tc.tile_pool(name="wq", bufs=1) as wp,
        tc.tile_pool(name="xt", bufs=3) as xp,
        tc.tile_pool(name="rope", bufs=2) as rp,
        tc.tile_pool(name="p1ps", bufs=1, space="PSUM") as pp1,
    ):
        cos_sb = wp.tile([64, S], F32)
        sin_sb = wp.tile([64, S], F32)
        vT = wp.tile([128, S], F32)
        ident_sb = wp.tile([128, 128], F32)
        ones_f = wp.tile([128, 128], F32)
        nc.vector.memset(ones_f, 1.0)
        nc.vector.tensor_copy(ones_sb, ones_f)
        make_identity(nc, ident_sb)

        wq3 = wqkv.rearrange("(k p) d -> p k d", p=128)
        x3 = xT.rearrange("(k p) s -> p k s", p=128)
        # x chunk (t=0, kg=0) first so the first matmul starts almost
        # immediately; weight k-tiles follow in per-tile DMAs
        w_sb = [wp.tile([128, DOUT], F32R, name=f"w{k}", tag=f"w{k}")
                for k in range(KT)]
        xg0 = xp.tile([128, KG, TCH], F32R, name="xg")
        nc.sync.dma_start(out=xg0, in_=x3[:, 0:KG, 0:TCH])
        for k in range(KT):
            nc.sync.dma_start(out=w_sb[k], in_=wq3[:, k, :])
        nc.sync.dma_start(out=cos_sb, in_=cos_t)
        nc.sync.dma_start(out=sin_sb, in_=sin_t)
        for t in range(NTCH):
            ps = [pp1.tile([128, TCH], F32, name=f"p1_{m}", tag=f"p1_{m}")
                  for m in range(MT)]
            for kg in range(KT // KG):
                if t == 0 and kg == 0:
                    xg = xg0
                else:
                    xg = xp.tile([128, KG, TCH], F32R, name="xg")
                    nc.sync.dma_start(
                        out=xg,
                        in_=x3[:, kg * KG:(kg + 1) * KG, t * TCH:(t + 1) * TCH])
                for ki in range(KG):
                    k = kg * KG + ki
                    for m in range(MT):
                        nc.tensor.matmul(
                            ps[m],
                            lhsT=w_sb[k][:, m * 128:(m + 1) * 128],
                            rhs=xg[:, ki, :],
                            start=(k == 0), stop=(k == KT - 1),
                        )
            cs = cos_sb[:, t * TCH:(t + 1) * TCH]
            sn = sin_sb[:, t * TCH:(t + 1) * TCH]
            for h in range(QH):
                _rope(nc, rp, ps[h], qT[:, h, t * TCH:(t + 1) * TCH], cs, sn)
            _rope(nc, rp, ps[QH], kT[:, t * TCH:(t + 1) * TCH], cs, sn)
            nc.scalar.copy(out=vT[:, t * TCH:(t + 1) * TCH], in_=ps[QH + 1])
            # V = vT.T for this chunk's kv tiles (PE transpose [d,tok]->[tok,d])
            for j in range(4 * t, 4 * t + 4):
                pv = pp1.tile([128, 128], F32, name="pvt", tag="pvt")
                nc.tensor.transpose(pv, vT[:, j * 128:(j + 1) * 128], ident_sb)
                nc.scalar.copy(out=V[:, j, :], in_=pv)

    # ---- phases 2+3: attention heads with per-head AllGather; o_proj for
    # head 0 interleaved into attention head 3, rest at the tail ---------
    with (
        tc.tile_pool(name="pt", bufs=8) as ptp,
        tc.tile_pool(name="ao", bufs=2) as aop,
        tc.tile_pool(name="ow", bufs=16) as owp,
        tc.tile_pool(name="at", bufs=2) as atp,
        tc.tile_pool(name="acc", bufs=1) as accp,
        tc.tile_pool(name="p2sc", bufs=2, space="PSUM") as pp2,
        tc.tile_pool(name="p2acc", bufs=1, space="PSUM") as pa2,
        tc.tile_pool(name="p3ps", bufs=1, space="PSUM") as pp3,
    ):
        # o_proj output accumulator: acc[:, b, :] = out rows b*128:(b+1)*128
        acc = accp.tile([128, S // 128, TCH], F32)
        ow3 = ow.rearrange("(k p) d -> p k d", p=128)
        ag3 = [attn_gat[h].rearrange("(r p) s -> p r s", p=128)
               for h in range(QH)]

        def attention_chunk(h, c):
            """One (head, q-chunk): scores+exp in j-pair waves, then PV."""
            jmax = 4 * c + 3
            po = pa2.tile([128, TCH], F32, name="po", tag="po")
            psum_s = pa2.tile([128, TCH], F32, name="ps", tag="ps")
            qslice = qT[:, h, c * TCH:(c + 1) * TCH]
            npair = (jmax + 1) // 2
            PW = 4  # j-pairs per pass-A/B wave (bounds live pt tiles)
            for p0 in range(0, npair, PW):
                p1 = min(p0 + PW, npair)
                pts = []
                for p in range(p0, p1):
                    # two score matmuls into one 2-bank psum tile, one exp
                    sc = pp2.tile([128, 2, TCH], F32, name="sc", tag="sc")
                    for i in range(2):
                        j = 2 * p + i
                        nc.tensor.matmul(
                            sc[:, i, :], lhsT=kT[:, j * 128:(j + 1) * 128],
                            rhs=qslice, start=True, stop=True)
                    pt = ptp.tile([128, 2, TCH], F32R, name="pt", tag="pt")
                    nc.scalar.activation(pt, sc, EXP, scale=SCALE)
                    for i in range(2):
                        j = 2 * p + i
                        rdiag = j - 4 * c
                        if rdiag >= 0:  # tile touches the causal diagonal
                            off = 384 - rdiag * 128
                            nc.vector.tensor_mul(
                                pt[:, i, :], pt[:, i, :],
                                stair_sb[:, off:off + TCH])
                    pts.append(pt)
                for idx, p in enumerate(range(p0, p1)):
                    for i in range(2):
                        j = 2 * p + i
                        nc.tensor.matmul(po, lhsT=V[:, j, :],
                                         rhs=pts[idx][:, i, :],
                                         start=(j == 0), stop=(j == jmax))
                        nc.tensor.matmul(psum_s, lhsT=ones_sb,
                                         rhs=pts[idx][:, i, :],
                                         start=(j == 0), stop=(j == jmax))
            rec = aop.tile([128, TCH], F32, name="rec")
            nc.vector.reciprocal(rec, psum_s)
            ao = aop.tile([128, TCH], F32, name="ao")
            nc.vector.tensor_mul(ao, po, rec)
            nc.sync.dma_start(
                out=attn_loc[h][:, c * TCH:(c + 1) * TCH], in_=ao)

        def allgather_head(h):
            nc.gpsimd.collective_compute(
                "AllGather",
                mybir.AluOpType.bypass,
                ins=[attn_loc[h][:, :]],
                outs=[attn_gat[h][:, :]],
                replica_groups=[list(range(NCORES))],
            )

        def oproj_load_weights(h):
            ows = []
            for r in range(NCORES):
                owk = owp.tile([128, DQ], F32R, name="owk", tag="owk")
                nc.sync.dma_start(out=owk, in_=ow3[:, r * QH + h, :])
                ows.append(owk)
            return ows

        def oproj_chunk(h, g, ows):
            """acc[:, 4g:4g+4, :] += sum_r at(r, h) @ ow(r, h) for 512 toks."""
            at = atp.tile([128, NCORES, TCH], F32R, name="at", tag="at")
            nc.sync.dma_start(
                out=at, in_=ag3[h][:, :, g * TCH:(g + 1) * TCH].bitcast(F32R))
            for mp in range(2):
                pcs = [pp3.tile([128, TCH], F32, name=f"pc{i}", tag=f"pc{i}")
                       for i in range(2)]
                for r in range(NCORES):
                    for i, mi in enumerate((2 * mp, 2 * mp + 1)):
                        nc.tensor.matmul(
                            pcs[i],
                            lhsT=at[:, r, mi * 128:(mi + 1) * 128],
                            rhs=ows[r],
                            start=(r == 0), stop=(r == NCORES - 1),
                        )
                for i, mi in enumerate((2 * mp, 2 * mp + 1)):
                    b = g * 4 + mi
                    if h == 0:
                        nc.scalar.copy(out=acc[:, b, :], in_=pcs[i])
                    else:
                        nc.vector.tensor_add(acc[:, b, :], acc[:, b, :],
                                             pcs[i])

        for h in range(3):
            for c in range(NTCH):
                attention_chunk(h, c)
            allgather_head(h)
        # head 3 attention interleaved with o_proj of the gathered head 0
        ows0 = oproj_load_weights(0)
        for c in range(NTCH):
            attention_chunk(3, c)
            oproj_chunk(0, c, ows0)
        allgather_head(3)
        for h in range(1, QH):
            ows = oproj_load_weights(h)
            for g in range(S // TCH):
                oproj_chunk(h, g, ows)

        nc.sync.dma_start(out=out.rearrange("(b p) d -> p b d", p=128), in_=acc)


_NC_CACHE = None


def build_program():
    global _NC_CACHE
    if _NC_CACHE is not None:
        return _NC_CACHE
    nc = bacc.Bacc("TRN2", target_bir_lowering=False, debug=False,
                   num_devices=NCORES)
    ins = {
        "xT": nc.dram_tensor("xT", [HIDDEN, S], F32R, kind="ExternalInput").ap(),
        "wqkv": nc.dram_tensor("wqkv", [HIDDEN, DOUT], F32R,
                               kind="ExternalInput").ap(),
        "ow": nc.dram_tensor("ow", [HIDDEN, DQ], F32R, kind="ExternalInput").ap(),
        "cos_t": nc.dram_tensor("cos_t", [64, S], F32, kind="ExternalInput").ap(),
        "sin_t": nc.dram_tensor("sin_t", [64, S], F32, kind="ExternalInput").ap(),
        "stair": nc.dram_tensor("stair", [128, 896], F32,
                                kind="ExternalInput").ap(),
    }
    outs = {"out": nc.dram_tensor("out", [S, DQ], F32, kind="ExternalOutput").ap()}
    with tile.TileContext(nc) as tc:
        with ExitStack() as ctx:
            build_kernel_body(ctx, tc, outs, ins)
    nc.compile()
    _NC_CACHE = nc
    return nc


def make_in_maps(hidden_states, position_ids, q_w, k_w, v_w, o_w):
    x = np.asarray(hidden_states, dtype=np.float32).reshape(S, HIDDEN)
    xT = np.ascontiguousarray(x.T)
    pos = np.asarray(position_ids).reshape(S).astype(np.float64)
    inv = 1.0 / (THETA ** (np.arange(0, HD, 2, dtype=np.float64) / HD))
    fr = inv[:, None] * pos[None, :]                       # [64, S]
    cos_t = np.cos(fr).astype(np.float32)
    sin_t = np.sin(fr).astype(np.float32)
    u = np.arange(896, dtype=np.int64)[None, :]
    kvi = np.arange(128, dtype=np.int64)[:, None]
    stair = ((u - kvi) >= 384).astype(np.float32)          # [128, 896]

    q_w = np.asarray(q_w, dtype=np.float32)
    k_w = np.asarray(k_w, dtype=np.float32)
    v_w = np.asarray(v_w, dtype=np.float32)
    o_w = np.asarray(o_w, dtype=np.float32)

    in_maps = []
    for c in range(NCORES):
        wqkv = np.ascontiguousarray(np.concatenate(
            [q_w[:, c * DQ:(c + 1) * DQ],
             k_w[:, c * HD:(c + 1) * HD],
             v_w[:, c * HD:(c + 1) * HD]], axis=1))
        owc = np.ascontiguousarray(o_w[:, c * DQ:(c + 1) * DQ])
        in_maps.append({"xT": xT, "wqkv": wqkv, "ow": owc,
                        "cos_t": cos_t, "sin_t": sin_t, "stair": stair})
    return in_maps


def run(inputs: dict, trace: bool = False):
    """Run on the 8 NeuronCores; returns (full_output, BassKernelResults)."""
    nc = build_program()
    in_maps = make_in_maps(**inputs)
    res = run_bass_kernel_spmd(nc, in_maps, core_ids=list(range(NCORES)),
                               trace=trace)
    full = np.concatenate([res.results[c]["out"] for c in range(NCORES)], axis=1)
    return full.reshape(1, S, HIDDEN), res


def kernel(**inputs) -> np.ndarray:
    out, _ = run(inputs)
    return out



# revision 13
# speedup vs baseline: 1.2113x; 1.2113x over previous
# Mistral sliding-window attention (B=1, S=2048, H=4096, 32 q heads / 8 kv
# heads, window 4096 -> plain causal at this S) on 8 Trainium2 NeuronCores.
#
# Sharding: tensor-parallel over heads. Core c owns q heads 4c..4c+3 and kv
# head c. hidden_states is replicated (transposed on host to [H, S] so the
# contraction dim is the partition dim). Each core computes its attention
# output slice attn.T [512, S]; per-head AllGathers assemble the full
# [4096, S] while later heads still compute, and each core accumulates a
# 512-column slice of o_proj head-by-head; the host concatenates the 8
# column slices into the full output.
#
# All tensors are bf16 on the wire and in SBUF (fp32 accumulation in PSUM):
# bf16 halves HBM traffic AND enables Fast Weight Load on the PE (fp32
# stationaries load at 2 cycles/col with no FWL, which made LDWEIGHTS
# co-critical with the matmuls in the fp32r version of this kernel).
# Scores are computed transposed (S.T[kv, q]) so the P@V contraction needs
# no transposes of the probability tiles; softmax denominators come from an
# all-ones stationary matmul accumulated alongside P@V, and the causal mask
# is a host-precomputed staircase slice multiplied in after exp. Attention
# runs a rolling 2-deep software pipeline per (head, q-chunk): scores+exp
# for kv-pair p issue ahead of the PV/sum matmuls of pair p-2, so the PE
# never stalls on the ACT engine. o_proj for head h-2 interleaves with
# attention of head h to keep the PE dense through the collective tail.

from contextlib import ExitStack

import ml_dtypes
import numpy as np

import concourse.bacc as bacc
import concourse.bass as bass
import concourse.mybir as mybir
import concourse.tile as tile
from concourse.bass_utils import run_bass_kernel_spmd
from concourse.masks import make_identity

HIDDEN = 4096
NH = 32
NKV = 8
HD = 128
THETA = 10000.0
S = 2048
NCORES = 8

QH = NH // NCORES          # 4 q heads per core
DQ = QH * HD               # 512 (per-core q/attn width)
DOUT = DQ + 2 * HD         # 768 = q heads + k + v projection width
MT = DOUT // 128           # 6 projection m-tiles (0..3 q, 4 k, 5 v)
KT = HIDDEN // 128         # 32 contraction tiles
KG = 8                     # x-load group: k-tiles per DMA (1MB bf16)
TCH = 512                  # token chunk (matmul moving dim)
NTCH = S // TCH            # 4
KVT = S // 128             # 16 kv tiles
SCALE = 1.0 / float(np.sqrt(HD))

F32 = mybir.dt.float32
BF16 = mybir.dt.bfloat16
EXP = mybir.ActivationFunctionType.Exp


def _rope(nc, pool, src, dst, cs, sn):
    """dst = src*cos + rotate_half(src)*sin, in [d, tok] layout.

    src is [128, n] (PSUM f32); dst is [128, n] bf16; cs/sn are [64, n]
    (the two 64-row halves share frequencies). rotate_half: rows 0:64 get
    -src[64:128], rows 64:128 get src[0:64]. NOTE: a tensor_tensor op may
    not take two SBUF inputs at different base partitions (walrus
    NCC_IBIR297), so the cross-half products go through [64]-row tiles at
    base partition 0 with the PSUM operand carrying the partition offset.
    """
    top, bot = src[0:64, :], src[64:128, :]
    ta = pool.tile([64, TCH], BF16, name="rope_a")
    tb = pool.tile([64, TCH], BF16, name="rope_b")
    nc.vector.tensor_mul(ta, top, cs)
    nc.vector.tensor_mul(tb, bot, sn)
    nc.vector.tensor_sub(dst[0:64, :], ta, tb)
    nc.vector.tensor_mul(ta, bot, cs)
    nc.vector.tensor_mul(tb, top, sn)
    nc.vector.tensor_add(dst[64:128, :], ta, tb)


def build_kernel_body(ctx: ExitStack, tc: tile.TileContext, outs, ins):
    nc = tc.nc
    xT, wqkv, ow, cos_t, sin_t, stair = (
        ins["xT"], ins["wqkv"], ins["ow"], ins["cos_t"], ins["sin_t"], ins["stair"],
    )
    out = outs["out"]

    # per-head bounce + gather buffers so each head's AllGather can fire as
    # soon as that head's attention is done (overlaps comm with compute)
    attn_loc = [nc.dram_tensor(f"attn_loc{h}", [HD, S], BF16).ap()
                for h in range(QH)]
    attn_gat = [nc.dram_tensor(f"attn_gat{h}", [NCORES * HD, S], BF16,
                               addr_space="Shared").ap()
                for h in range(QH)]

    singles = ctx.enter_context(tc.tile_pool(name="singles", bufs=1))
    stair_sb = singles.tile([128, 896], BF16)
    ones_sb = singles.tile([128, 128], BF16)

    # persistent projection outputs, [d, tok] layout
    qT = singles.tile([128, QH, S], BF16)    # q head h -> qT[:, h, :]
    kT = singles.tile([128, S], BF16)
    V = singles.tile([128, KVT, HD], BF16)   # V[:, j, :] = [tok 128, d 128]

    # ---- phase 1: QKV projection + RoPE --------------------------------
    with (
        tc.tile_pool(name="wq", bufs=1) as wp,
        tc.tile_pool(name="xt", bufs=3) as xp,
        tc.tile_pool(name="rope", bufs=2) as rp,
        tc.tile_pool(name="p1ps", bufs=1, space="PSUM") as pp1,
    ):
        cos_sb = wp.tile([64, S], F32)
        sin_sb = wp.tile([64, S], F32)
        vT = wp.tile([128, S], BF16)
        ident_sb = wp.tile([128, 128], BF16)
        nc.vector.memset(ones_sb, 1.0)
        make_identity(nc, ident_sb)

        x3 = xT.rearrange("(k p) s -> p k s", p=128)
        # x chunk (t=0, kg=0) first so the first matmul starts almost
        # immediately; weight k-groups follow, then the small tensors
        w_sb = [wp.tile([128, KG, DOUT], BF16, name=f"w{g}", tag=f"w{g}")
                for g in range(KT // KG)]
        xg0 = xp.tile([128, KG, TCH], BF16, name="xg")
        nc.sync.dma_start(out=xg0, in_=x3[:, 0:KG, 0:TCH])
        wq4 = wqkv.rearrange("(g k p) d -> p g k d", p=128, k=KG)
        for g in range(KT // KG):
            nc.sync.dma_start(out=w_sb[g], in_=wq4[:, g, :, :])
        nc.sync.dma_start(out=cos_sb, in_=cos_t)
        nc.sync.dma_start(out=sin_sb, in_=sin_t)
        nc.sync.dma_start(out=stair_sb, in_=stair)
        for t in range(NTCH):
            ps = [pp1.tile([128, TCH], F32, name=f"p1_{m}", tag=f"p1_{m}")
                  for m in range(MT)]
            for kg in range(KT // KG):
                if t == 0 and kg == 0:
                    xg = xg0
                else:
                    xg = xp.tile([128, KG, TCH], BF16, name="xg")
                    nc.sync.dma_start(
                        out=xg,
                        in_=x3[:, kg * KG:(kg + 1) * KG, t * TCH:(t + 1) * TCH])
                for ki in range(KG):
                    k = kg * KG + ki
                    for m in range(MT):
                        nc.tensor.matmul(
                            ps[m],
                            lhsT=w_sb[kg][:, ki, m * 128:(m + 1) * 128],
                            rhs=xg[:, ki, :],
                            start=(k == 0), stop=(k == KT - 1),
                        )
            cs = cos_sb[:, t * TCH:(t + 1) * TCH]
            sn = sin_sb[:, t * TCH:(t + 1) * TCH]
            # k first: attention's score matmuls depend on kT, and the last
            # chunk's k-rope is the only thing gating attention c=3
            _rope(nc, rp, ps[QH], kT[:, t * TCH:(t + 1) * TCH], cs, sn)
            nc.scalar.copy(out=vT[:, t * TCH:(t + 1) * TCH], in_=ps[QH + 1])
            for h in range(QH):
                _rope(nc, rp, ps[h], qT[:, h, t * TCH:(t + 1) * TCH], cs, sn)
            # V = vT.T for this chunk's kv tiles (PE transpose [d,tok]->[tok,d])
            for j in range(4 * t, 4 * t + 4):
                pv = pp1.tile([128, 128], BF16, name="pvt", tag="pvt")
                nc.tensor.transpose(pv, vT[:, j * 128:(j + 1) * 128], ident_sb)
                nc.scalar.copy(out=V[:, j, :], in_=pv)

    # ---- phase 2: attention heads with per-head AllGather ---------------
    ow3 = ow.rearrange("(k p) d -> p k d", p=128)
    out3 = out.rearrange("(b p) d -> p b d", p=128)
    ag3 = [attn_gat[h].rearrange("(r p) s -> p r s", p=128)
           for h in range(QH)]
    owp = ctx.enter_context(tc.tile_pool(name="ow", bufs=32))

    with (
        tc.tile_pool(name="pt", bufs=4) as ptp,
        tc.tile_pool(name="ao", bufs=2) as aop,
        tc.tile_pool(name="p2sc", bufs=2, space="PSUM") as pp2,
        tc.tile_pool(name="p2acc", bufs=2, space="PSUM") as pa2,
    ):
        def attention_chunk(h, c):
            """One (head, q-chunk): rolling pipeline, PV trails scores by 2
            kv-pairs so the PE never waits on the ACT engine's exp."""
            jmax = 4 * c + 3
            npair = (jmax + 1) // 2
            po = pa2.tile([128, TCH], F32, name="po", tag="po")
            psum_s = pa2.tile([128, TCH], F32, name="ps", tag="ps")
            qslice = qT[:, h, c * TCH:(c + 1) * TCH]
            pts = {}

            def emit_scores(p):
                sc = pp2.tile([128, 2, TCH], F32, name="sc", tag="sc")
                for i in range(2):
                    j = 2 * p + i
                    nc.tensor.matmul(
                        sc[:, i, :], lhsT=kT[:, j * 128:(j + 1) * 128],
                        rhs=qslice, start=True, stop=True)
                pt = ptp.tile([128, 2, TCH], BF16, name="pt", tag="pt")
                nc.scalar.activation(pt, sc, EXP, scale=SCALE)
                for i in range(2):
                    j = 2 * p + i
                    rdiag = j - 4 * c
                    if rdiag >= 0:  # tile touches the causal diagonal
                        off = 384 - rdiag * 128
                        nc.vector.tensor_mul(
                            pt[:, i, :], pt[:, i, :],
                            stair_sb[:, off:off + TCH])
                pts[p] = pt

            def emit_pv(p):
                pt = pts.pop(p)
                for i in range(2):
                    j = 2 * p + i
                    nc.tensor.matmul(po, lhsT=V[:, j, :], rhs=pt[:, i, :],
                                     start=(j == 0), stop=(j == jmax))
                    nc.tensor.matmul(psum_s, lhsT=ones_sb, rhs=pt[:, i, :],
                                     start=(j == 0), stop=(j == jmax))

            for p in range(npair):
                emit_scores(p)
                if p >= 2:
                    emit_pv(p - 2)
            for p in (npair - 2, npair - 1):
                if p in pts:
                    emit_pv(p)

            rec = aop.tile([128, TCH], F32, name="rec")
            nc.vector.reciprocal(rec, psum_s)
            ao = aop.tile([128, TCH], BF16, name="ao")
            nc.vector.tensor_mul(ao, po, rec)
            nc.sync.dma_start(
                out=attn_loc[h][:, c * TCH:(c + 1) * TCH], in_=ao)

        def allgather_head(h):
            nc.gpsimd.collective_compute(
                "AllGather",
                mybir.AluOpType.bypass,
                ins=[attn_loc[h][:, :]],
                outs=[attn_gat[h][:, :]],
                replica_groups=[list(range(NCORES))],
            )

        for h in range(QH):
            for c in range(NTCH):
                attention_chunk(h, c)
            allgather_head(h)
            if h == 0:
                # o_proj weights DMA (4.2MB bf16) rides under attention
                ows = [[None] * NCORES for _ in range(QH)]
                for hh in range(QH):
                    for r in range(NCORES):
                        owk = owp.tile([128, DQ], BF16, name="owk", tag="owk")
                        nc.sync.dma_start(out=owk, in_=ow3[:, r * QH + hh, :])
                        ows[hh][r] = owk

    # ---- phase 3: o_proj, accumulated over all (head, rank) in PSUM -----
    with (
        tc.tile_pool(name="at", bufs=8) as atp,
        tc.tile_pool(name="stg", bufs=2) as stp,
        tc.tile_pool(name="p3ps", bufs=2, space="PSUM") as pp3,
    ):
        for g in range(S // TCH):
            ats = []
            for h in range(QH):
                at = atp.tile([128, NCORES, TCH], BF16, name="at", tag="at")
                nc.sync.dma_start(
                    out=at, in_=ag3[h][:, :, g * TCH:(g + 1) * TCH])
                ats.append(at)
            pc = pp3.tile([128, 4, TCH], F32, name="pc", tag="pc")
            for h in range(QH):
                for r in range(NCORES):
                    for mi in range(4):
                        nc.tensor.matmul(
                            pc[:, mi, :],
                            lhsT=ats[h][:, r, mi * 128:(mi + 1) * 128],
                            rhs=ows[h][r],
                            start=(h == 0 and r == 0),
                            stop=(h == QH - 1 and r == NCORES - 1),
                        )
            stg = stp.tile([128, 4, TCH], F32, name="stg")
            nc.scalar.copy(out=stg, in_=pc)
            nc.sync.dma_start(out=out3[:, 4 * g:4 * g + 4, :], in_=stg)


_NC_CACHE = None


def build_program():
    global _NC_CACHE
    if _NC_CACHE is not None:
        return _NC_CACHE
    nc = bacc.Bacc("TRN2", target_bir_lowering=False, debug=False,
                   num_devices=NCORES)
    ins = {
        "xT": nc.dram_tensor("xT", [HIDDEN, S], BF16, kind="ExternalInput").ap(),
        "wqkv": nc.dram_tensor("wqkv", [HIDDEN, DOUT], BF16,
                               kind="ExternalInput").ap(),
        "ow": nc.dram_tensor("ow", [HIDDEN, DQ], BF16, kind="ExternalInput").ap(),
        "cos_t": nc.dram_tensor("cos_t", [64, S], F32, kind="ExternalInput").ap(),
        "sin_t": nc.dram_tensor("sin_t", [64, S], F32, kind="ExternalInput").ap(),
        "stair": nc.dram_tensor("stair", [128, 896], BF16,
                                kind="ExternalInput").ap(),
    }
    outs = {"out": nc.dram_tensor("out", [S, DQ], F32, kind="ExternalOutput").ap()}
    with tile.TileContext(nc) as tc:
        with ExitStack() as ctx:
            build_kernel_body(ctx, tc, outs, ins)
    nc.compile()
    _NC_CACHE = nc
    return nc


def make_in_maps(hidden_states, position_ids, q_w, k_w, v_w, o_w):
    bf16 = ml_dtypes.bfloat16
    x = np.asarray(hidden_states, dtype=np.float32).reshape(S, HIDDEN)
    xT = np.ascontiguousarray(x.T).astype(bf16)
    pos = np.asarray(position_ids).reshape(S).astype(np.float64)
    inv = 1.0 / (THETA ** (np.arange(0, HD, 2, dtype=np.float64) / HD))
    fr = inv[:, None] * pos[None, :]                       # [64, S]
    cos_t = np.cos(fr).astype(np.float32)
    sin_t = np.sin(fr).astype(np.float32)
    u = np.arange(896, dtype=np.int64)[None, :]
    kvi = np.arange(128, dtype=np.int64)[:, None]
    stair = ((u - kvi) >= 384).astype(bf16)                # [128, 896]

    q_w = np.asarray(q_w, dtype=np.float32)
    k_w = np.asarray(k_w, dtype=np.float32)
    v_w = np.asarray(v_w, dtype=np.float32)
    o_w = np.asarray(o_w, dtype=np.float32)

    in_maps = []
    for c in range(NCORES):
        wqkv = np.ascontiguousarray(np.concatenate(
            [q_w[:, c * DQ:(c + 1) * DQ],
             k_w[:, c * HD:(c + 1) * HD],
             v_w[:, c * HD:(c + 1) * HD]], axis=1)).astype(bf16)
        owc = np.ascontiguousarray(o_w[:, c * DQ:(c + 1) * DQ]).astype(bf16)
        in_maps.append({"xT": xT, "wqkv": wqkv, "ow": owc,
                        "cos_t": cos_t, "sin_t": sin_t, "stair": stair})
    return in_maps


def run(inputs: dict, trace: bool = False):
    """Run on the 8 NeuronCores; returns (full_output, BassKernelResults)."""
    nc = build_program()
    in_maps = make_in_maps(**inputs)
    res = run_bass_kernel_spmd(nc, in_maps, core_ids=list(range(NCORES)),
                               trace=trace)
    full = np.concatenate([res.results[c]["out"] for c in range(NCORES)], axis=1)
    return full.reshape(1, S, HIDDEN), res


def kernel(**inputs) -> np.ndarray:
    out, _ = run(inputs)
    return out


# revision 18
# speedup vs baseline: 1.3024x; 1.0752x over previous
# Mistral sliding-window attention (B=1, S=2048, H=4096, 32 q heads / 8 kv
# heads, window 4096 -> plain causal at this S) on 8 Trainium2 NeuronCores.
#
# Sharding: tensor-parallel over heads. Core c owns q heads 4c..4c+3 and kv
# head c. hidden_states is replicated (transposed on host to [H, S] so the
# contraction dim is the partition dim). Each core computes its attention
# output slice attn.T [512, S]; per-head AllGathers assemble the full
# [4096, S] while later heads still compute, and each core accumulates a
# 512-column slice of o_proj head-by-head; the host concatenates the 8
# column slices into the full output.
#
# All tensors are bf16 on the wire and in SBUF (fp32 accumulation in PSUM):
# bf16 halves HBM traffic AND enables Fast Weight Load on the PE (fp32
# stationaries load at 2 cycles/col with no FWL, which made LDWEIGHTS
# co-critical with the matmuls in the fp32r version of this kernel).
# Scores are computed transposed (S.T[kv, q]) so the P@V contraction needs
# no transposes of the probability tiles; softmax denominators come from an
# all-ones stationary matmul accumulated alongside P@V, and the causal mask
# is a host-precomputed staircase slice multiplied in after exp. Attention
# runs a rolling 2-deep software pipeline per (head, q-chunk): scores+exp
# for kv-pair p issue ahead of the PV/sum matmuls of pair p-2, so the PE
# never stalls on the ACT engine. o_proj for head h-2 interleaves with
# attention of head h to keep the PE dense through the collective tail.

from contextlib import ExitStack

import ml_dtypes
import numpy as np

import concourse.bacc as bacc
import concourse.bass as bass
import concourse.mybir as mybir
import concourse.tile as tile
from concourse.bass_utils import run_bass_kernel_spmd
from concourse.masks import make_identity

HIDDEN = 4096
NH = 32
NKV = 8
HD = 128
THETA = 10000.0
S = 2048
NCORES = 8

QH = NH // NCORES          # 4 q heads per core
DQ = QH * HD               # 512 (per-core q/attn width)
DOUT = DQ + 2 * HD         # 768 = q heads + k + v projection width
MT = DOUT // 128           # 6 projection m-tiles (0..3 q, 4 k, 5 v)
KT = HIDDEN // 128         # 32 contraction tiles
KG = 8                     # x-load group: k-tiles per DMA (1MB bf16)
TCH = 512                  # token chunk (matmul moving dim)
NTCH = S // TCH            # 4
KVT = S // 128             # 16 kv tiles
SCALE = 1.0 / float(np.sqrt(HD))

F32 = mybir.dt.float32
BF16 = mybir.dt.bfloat16
EXP = mybir.ActivationFunctionType.Exp


def _rope(nc, pool, src, dst, cs, sn):
    """dst = src*cos + rotate_half(src)*sin, in [d, tok] layout.

    src is [128, n] (PSUM f32); dst is [128, n] bf16; cs/sn are [64, n]
    (the two 64-row halves share frequencies). rotate_half: rows 0:64 get
    -src[64:128], rows 64:128 get src[0:64]. NOTE: a tensor_tensor op may
    not take two SBUF inputs at different base partitions (walrus
    NCC_IBIR297), so the cross-half products go through [64]-row tiles at
    base partition 0 with the PSUM operand carrying the partition offset.
    """
    top, bot = src[0:64, :], src[64:128, :]
    ta = pool.tile([64, TCH], BF16, name="rope_a")
    tb = pool.tile([64, TCH], BF16, name="rope_b")
    nc.vector.tensor_mul(ta, top, cs)
    nc.vector.tensor_mul(tb, bot, sn)
    nc.vector.tensor_sub(dst[0:64, :], ta, tb)
    nc.vector.tensor_mul(ta, bot, cs)
    nc.vector.tensor_mul(tb, top, sn)
    nc.vector.tensor_add(dst[64:128, :], ta, tb)


def build_kernel_body(ctx: ExitStack, tc: tile.TileContext, outs, ins):
    nc = tc.nc
    xT, wqkv, ow, cos_t, sin_t, stair = (
        ins["xT"], ins["wqkv"], ins["ow"], ins["cos_t"], ins["sin_t"], ins["stair"],
    )
    out = outs["out"]

    # per-head bounce + gather buffers so each head's AllGather can fire as
    # soon as that head's attention is done (overlaps comm with compute).
    # Head 3 (the last one computed) gathers per q-chunk instead, so its
    # final AllGather only covers 512 tokens and o_proj isn't gated on a
    # full-head collective at the tail.
    attn_loc = [nc.dram_tensor(f"attn_loc{h}", [HD, S], BF16).ap()
                for h in range(QH - 1)]
    attn_gat = [nc.dram_tensor(f"attn_gat{h}", [NCORES * HD, S], BF16,
                               addr_space="Shared").ap()
                for h in range(QH - 1)]
    attn_loc3 = [nc.dram_tensor(f"attn_loc3_{c}", [HD, TCH], BF16).ap()
                 for c in range(NTCH)]
    attn_gat3 = [nc.dram_tensor(f"attn_gat3_{c}", [NCORES * HD, TCH], BF16,
                                addr_space="Shared").ap()
                 for c in range(NTCH)]

    singles = ctx.enter_context(tc.tile_pool(name="singles", bufs=1))
    stair_sb = singles.tile([128, 896], BF16)
    ones_sb = singles.tile([128, 128], BF16)

    # persistent projection outputs, [d, tok] layout
    qT = singles.tile([128, QH, S], BF16)    # q head h -> qT[:, h, :]
    kT = singles.tile([128, S], BF16)
    V = singles.tile([128, KVT, HD], BF16)   # V[:, j, :] = [tok 128, d 128]

    # ---- phase 1: QKV projection + RoPE --------------------------------
    with (
        tc.tile_pool(name="wq", bufs=1) as wp,
        tc.tile_pool(name="xt", bufs=3) as xp,
        tc.tile_pool(name="rope", bufs=2) as rp,
        tc.tile_pool(name="p1ps", bufs=1, space="PSUM") as pp1,
    ):
        cos_sb = wp.tile([64, S], F32)
        sin_sb = wp.tile([64, S], F32)
        vT = wp.tile([128, S], BF16)
        ident_sb = wp.tile([128, 128], BF16)
        nc.vector.memset(ones_sb, 1.0)
        make_identity(nc, ident_sb)

        x3 = xT.rearrange("(k p) s -> p k s", p=128)
        wq4 = wqkv.rearrange("(g k p) d -> p g k d", p=128, k=KG)
        # DMA order: the first two k-tiles of weights+x land first so the
        # first matmul starts ~2us in; after that, weight groups interleave
        # just-in-time with chunk-0 x groups so x loads aren't queued behind
        # the whole 6.3MB weight stream.
        w00 = wp.tile([128, 2, DOUT], BF16, name="w00", tag="w00")
        w01 = wp.tile([128, KG - 2, DOUT], BF16, name="w01", tag="w01")
        w_sb = [None] + [wp.tile([128, KG, DOUT], BF16, name=f"w{g}",
                                 tag=f"w{g}")
                         for g in range(1, KT // KG)]
        x00 = xp.tile([128, 2, TCH], BF16, name="x00", tag="x00")
        x01 = xp.tile([128, KG - 2, TCH], BF16, name="x01", tag="x01")
        nc.sync.dma_start(out=w00, in_=wq4[:, 0, 0:2, :])
        nc.sync.dma_start(out=x00, in_=x3[:, 0:2, 0:TCH])
        nc.sync.dma_start(out=w01, in_=wq4[:, 0, 2:KG, :])
        nc.sync.dma_start(out=x01, in_=x3[:, 2:KG, 0:TCH])

        def wslice(kg, ki, m):
            if kg == 0:
                wt, i = (w00, ki) if ki < 2 else (w01, ki - 2)
            else:
                wt, i = w_sb[kg], ki
            return wt[:, i, m * 128:(m + 1) * 128]

        xg_t0 = [None] * (KT // KG)
        for g in range(1, KT // KG):
            nc.sync.dma_start(out=w_sb[g], in_=wq4[:, g, :, :])
            xg = xp.tile([128, KG, TCH], BF16, name="xg")
            nc.sync.dma_start(
                out=xg, in_=x3[:, g * KG:(g + 1) * KG, 0:TCH])
            xg_t0[g] = xg
        nc.sync.dma_start(out=cos_sb, in_=cos_t)
        nc.sync.dma_start(out=sin_sb, in_=sin_t)
        nc.sync.dma_start(out=stair_sb, in_=stair)

        def chunk_epilogue_v(t):
            # V = vT.T for this chunk's kv tiles (PE transpose [d,tok]->[tok,d])
            for j in range(4 * t, 4 * t + 4):
                pv = pp1.tile([128, 128], BF16, name="pvt", tag="pvt")
                nc.tensor.transpose(pv, vT[:, j * 128:(j + 1) * 128], ident_sb)
                nc.scalar.copy(out=V[:, j, :], in_=pv)

        for t in range(NTCH - 1):
            ps = [pp1.tile([128, TCH], F32, name=f"p1_{m}", tag=f"p1_{m}")
                  for m in range(MT)]
            for kg in range(KT // KG):
                if t == 0:
                    xg = xg_t0[kg]  # None for kg==0: wslice/x00/x01 cover it
                else:
                    xg = xp.tile([128, KG, TCH], BF16, name="xg")
                    nc.sync.dma_start(
                        out=xg,
                        in_=x3[:, kg * KG:(kg + 1) * KG, t * TCH:(t + 1) * TCH])
                for ki in range(KG):
                    k = kg * KG + ki
                    if t == 0 and kg == 0:
                        xs = x00[:, ki, :] if ki < 2 else x01[:, ki - 2, :]
                    else:
                        xs = xg[:, ki, :]
                    for m in range(MT):
                        nc.tensor.matmul(
                            ps[m], lhsT=wslice(kg, ki, m), rhs=xs,
                            start=(k == 0), stop=(k == KT - 1),
                        )
            cs = cos_sb[:, t * TCH:(t + 1) * TCH]
            sn = sin_sb[:, t * TCH:(t + 1) * TCH]
            # k first: attention's score matmuls depend on kT
            _rope(nc, rp, ps[QH], kT[:, t * TCH:(t + 1) * TCH], cs, sn)
            nc.scalar.copy(out=vT[:, t * TCH:(t + 1) * TCH], in_=ps[QH + 1])
            for h in range(QH):
                _rope(nc, rp, ps[h], qT[:, h, t * TCH:(t + 1) * TCH], cs, sn)
            chunk_epilogue_v(t)

        # Last chunk runs m-outer / k-inner from a fully-prefetched x slab:
        # each projection finishes 32 matmuls apart, so its RoPE overlaps
        # the remaining matmul stream instead of draining after phase 1 —
        # attention starts ~immediately at the seam.
        t = NTCH - 1
        xbig = wp.tile([128, KT, TCH], BF16, name="xbig", tag="xbig")
        for g in range(KT // KG):
            nc.sync.dma_start(
                out=xbig[:, g * KG:(g + 1) * KG, :],
                in_=x3[:, g * KG:(g + 1) * KG, t * TCH:(t + 1) * TCH])
        ps = [pp1.tile([128, TCH], F32, name=f"p1_{m}", tag=f"p1_{m}")
              for m in range(MT)]
        cs = cos_sb[:, t * TCH:(t + 1) * TCH]
        sn = sin_sb[:, t * TCH:(t + 1) * TCH]
        for m in (QH, 0, 1, 2, 3, QH + 1):
            for k in range(KT):
                nc.tensor.matmul(
                    ps[m], lhsT=wslice(k // KG, k % KG, m),
                    rhs=xbig[:, k, :], start=(k == 0), stop=(k == KT - 1),
                )
            if m == QH:
                _rope(nc, rp, ps[m], kT[:, t * TCH:(t + 1) * TCH], cs, sn)
            elif m < QH:
                _rope(nc, rp, ps[m], qT[:, m, t * TCH:(t + 1) * TCH], cs, sn)
            else:
                nc.scalar.copy(out=vT[:, t * TCH:(t + 1) * TCH], in_=ps[m])
        chunk_epilogue_v(t)

    # ---- phase 2: attention heads with per-head AllGather ---------------
    ow3 = ow.rearrange("(k p) d -> p k d", p=128)
    out3 = out.rearrange("(b p) d -> p b d", p=128)
    ag3 = [attn_gat[h].rearrange("(r p) s -> p r s", p=128)
           for h in range(QH - 1)]
    ag3c = [attn_gat3[c].rearrange("(r p) s -> p r s", p=128)
            for c in range(NTCH)]
    owp = ctx.enter_context(tc.tile_pool(name="ow", bufs=32))

    with (
        tc.tile_pool(name="pt", bufs=4) as ptp,
        tc.tile_pool(name="ao", bufs=2) as aop,
        tc.tile_pool(name="p2sc", bufs=2, space="PSUM") as pp2,
        tc.tile_pool(name="p2acc", bufs=2, space="PSUM") as pa2,
    ):
        def attention_chunk(h, c):
            """One (head, q-chunk): rolling pipeline, PV trails scores by 2
            kv-pairs so the PE never waits on the ACT engine's exp."""
            jmax = 4 * c + 3
            npair = (jmax + 1) // 2
            po = pa2.tile([128, TCH], F32, name="po", tag="po")
            psum_s = pa2.tile([128, TCH], F32, name="ps", tag="ps")
            qslice = qT[:, h, c * TCH:(c + 1) * TCH]
            pts = {}

            def emit_scores(p):
                sc = pp2.tile([128, 2, TCH], F32, name="sc", tag="sc")
                for i in range(2):
                    j = 2 * p + i
                    nc.tensor.matmul(
                        sc[:, i, :], lhsT=kT[:, j * 128:(j + 1) * 128],
                        rhs=qslice, start=True, stop=True)
                pt = ptp.tile([128, 2, TCH], BF16, name="pt", tag="pt")
                nc.scalar.activation(pt, sc, EXP, scale=SCALE)
                for i in range(2):
                    j = 2 * p + i
                    rdiag = j - 4 * c
                    if rdiag >= 0:  # tile touches the causal diagonal
                        off = 384 - rdiag * 128
                        nc.vector.tensor_mul(
                            pt[:, i, :], pt[:, i, :],
                            stair_sb[:, off:off + TCH])
                pts[p] = pt

            def emit_pv(p):
                pt = pts.pop(p)
                for i in range(2):
                    j = 2 * p + i
                    nc.tensor.matmul(po, lhsT=V[:, j, :], rhs=pt[:, i, :],
                                     start=(j == 0), stop=(j == jmax))
                    nc.tensor.matmul(psum_s, lhsT=ones_sb, rhs=pt[:, i, :],
                                     start=(j == 0), stop=(j == jmax))

            for p in range(npair):
                emit_scores(p)
                if p >= 2:
                    emit_pv(p - 2)
            for p in (npair - 2, npair - 1):
                if p in pts:
                    emit_pv(p)

            rec = aop.tile([128, TCH], F32, name="rec")
            nc.vector.reciprocal(rec, psum_s)
            ao = aop.tile([128, TCH], BF16, name="ao")
            nc.vector.tensor_mul(ao, po, rec)
            if h == QH - 1:
                nc.sync.dma_start(out=attn_loc3[c][:, :], in_=ao)
            else:
                nc.sync.dma_start(
                    out=attn_loc[h][:, c * TCH:(c + 1) * TCH], in_=ao)

        def allgather(ins_ap, outs_ap):
            nc.gpsimd.collective_compute(
                "AllGather",
                mybir.AluOpType.bypass,
                ins=[ins_ap],
                outs=[outs_ap],
                replica_groups=[list(range(NCORES))],
            )

        for h in range(QH):
            for c in range(NTCH):
                attention_chunk(h, c)
                if h == QH - 1:
                    allgather(attn_loc3[c][:, :], attn_gat3[c][:, :])
            if h < QH - 1:
                allgather(attn_loc[h][:, :], attn_gat[h][:, :])
            if h == 0:
                # o_proj weights DMA (4.2MB bf16) rides under attention
                ows = [[None] * NCORES for _ in range(QH)]
                for hh in range(QH):
                    for r in range(NCORES):
                        owk = owp.tile([128, DQ], BF16, name="owk", tag="owk")
                        nc.sync.dma_start(out=owk, in_=ow3[:, r * QH + hh, :])
                        ows[hh][r] = owk

    # ---- phase 3: o_proj, accumulated over all (head, rank) in PSUM -----
    with (
        tc.tile_pool(name="at", bufs=8) as atp,
        tc.tile_pool(name="stg", bufs=2) as stp,
        tc.tile_pool(name="p3ps", bufs=2, space="PSUM") as pp3,
    ):
        for g in range(S // TCH):
            ats = []
            for h in range(QH):
                at = atp.tile([128, NCORES, TCH], BF16, name="at", tag="at")
                if h == QH - 1:
                    nc.sync.dma_start(out=at, in_=ag3c[g][:, :, :])
                else:
                    nc.sync.dma_start(
                        out=at, in_=ag3[h][:, :, g * TCH:(g + 1) * TCH])
                ats.append(at)
            pc = pp3.tile([128, 4, TCH], F32, name="pc", tag="pc")
            for h in range(QH):
                for r in range(NCORES):
                    for mi in range(4):
                        nc.tensor.matmul(
                            pc[:, mi, :],
                            lhsT=ats[h][:, r, mi * 128:(mi + 1) * 128],
                            rhs=ows[h][r],
                            start=(h == 0 and r == 0),
                            stop=(h == QH - 1 and r == NCORES - 1),
                        )
            stg = stp.tile([128, 4, TCH], F32, name="stg")
            nc.scalar.copy(out=stg, in_=pc)
            nc.sync.dma_start(out=out3[:, 4 * g:4 * g + 4, :], in_=stg)


_NC_CACHE = None


def build_program():
    global _NC_CACHE
    if _NC_CACHE is not None:
        return _NC_CACHE
    nc = bacc.Bacc("TRN2", target_bir_lowering=False, debug=False,
                   num_devices=NCORES)
    ins = {
        "xT": nc.dram_tensor("xT", [HIDDEN, S], BF16, kind="ExternalInput").ap(),
        "wqkv": nc.dram_tensor("wqkv", [HIDDEN, DOUT], BF16,
                               kind="ExternalInput").ap(),
        "ow": nc.dram_tensor("ow", [HIDDEN, DQ], BF16, kind="ExternalInput").ap(),
        "cos_t": nc.dram_tensor("cos_t", [64, S], F32, kind="ExternalInput").ap(),
        "sin_t": nc.dram_tensor("sin_t", [64, S], F32, kind="ExternalInput").ap(),
        "stair": nc.dram_tensor("stair", [128, 896], BF16,
                                kind="ExternalInput").ap(),
    }
    outs = {"out": nc.dram_tensor("out", [S, DQ], F32, kind="ExternalOutput").ap()}
    with tile.TileContext(nc) as tc:
        with ExitStack() as ctx:
            build_kernel_body(ctx, tc, outs, ins)
    nc.compile()
    _NC_CACHE = nc
    return nc


def make_in_maps(hidden_states, position_ids, q_w, k_w, v_w, o_w):
    bf16 = ml_dtypes.bfloat16
    x = np.asarray(hidden_states, dtype=np.float32).reshape(S, HIDDEN)
    xT = np.ascontiguousarray(x.T).astype(bf16)
    pos = np.asarray(position_ids).reshape(S).astype(np.float64)
    inv = 1.0 / (THETA ** (np.arange(0, HD, 2, dtype=np.float64) / HD))
    fr = inv[:, None] * pos[None, :]                       # [64, S]
    cos_t = np.cos(fr).astype(np.float32)
    sin_t = np.sin(fr).astype(np.float32)
    u = np.arange(896, dtype=np.int64)[None, :]
    kvi = np.arange(128, dtype=np.int64)[:, None]
    stair = ((u - kvi) >= 384).astype(bf16)                # [128, 896]

    q_w = np.asarray(q_w, dtype=np.float32)
    k_w = np.asarray(k_w, dtype=np.float32)
    v_w = np.asarray(v_w, dtype=np.float32)
    o_w = np.asarray(o_w, dtype=np.float32)

    in_maps = []
    for c in range(NCORES):
        wqkv = np.ascontiguousarray(np.concatenate(
            [q_w[:, c * DQ:(c + 1) * DQ],
             k_w[:, c * HD:(c + 1) * HD],
             v_w[:, c * HD:(c + 1) * HD]], axis=1)).astype(bf16)
        owc = np.ascontiguousarray(o_w[:, c * DQ:(c + 1) * DQ]).astype(bf16)
        in_maps.append({"xT": xT, "wqkv": wqkv, "ow": owc,
                        "cos_t": cos_t, "sin_t": sin_t, "stair": stair})
    return in_maps


def run(inputs: dict, trace: bool = False):
    """Run on the 8 NeuronCores; returns (full_output, BassKernelResults)."""
    nc = build_program()
    in_maps = make_in_maps(**inputs)
    res = run_bass_kernel_spmd(nc, in_maps, core_ids=list(range(NCORES)),
                               trace=trace)
    full = np.concatenate([res.results[c]["out"] for c in range(NCORES)], axis=1)
    return full.reshape(1, S, HIDDEN), res


def kernel(**inputs) -> np.ndarray:
    out, _ = run(inputs)
    return out


# revision 21
# speedup vs baseline: 1.3791x; 1.0589x over previous
# Mistral sliding-window attention (B=1, S=2048, H=4096, 32 q heads / 8 kv
# heads, window 4096 -> plain causal at this S) on 8 Trainium2 NeuronCores.
#
# Sharding: tensor-parallel over heads. Core c owns q heads 4c..4c+3 and kv
# head c. hidden_states is replicated (transposed on host to [H, S] so the
# contraction dim is the partition dim). Each core computes its attention
# output slice attn.T [512, S]; per-head AllGathers assemble the full
# [4096, S] while later heads still compute, and each core accumulates a
# 512-column slice of o_proj head-by-head; the host concatenates the 8
# column slices into the full output.
#
# All tensors are bf16 on the wire and in SBUF (fp32 accumulation in PSUM):
# bf16 halves HBM traffic AND enables Fast Weight Load on the PE (fp32
# stationaries load at 2 cycles/col with no FWL, which made LDWEIGHTS
# co-critical with the matmuls in the fp32r version of this kernel).
# Scores are computed transposed (S.T[kv, q]) so the P@V contraction needs
# no transposes of the probability tiles; softmax denominators come from an
# all-ones stationary matmul accumulated alongside P@V, and the causal mask
# is a host-precomputed staircase slice multiplied in after exp. Attention
# runs a rolling 2-deep software pipeline per (head, q-chunk): scores+exp
# for kv-pair p issue ahead of the PV/sum matmuls of pair p-2, so the PE
# never stalls on the ACT engine. o_proj for head h-2 interleaves with
# attention of head h to keep the PE dense through the collective tail.

from collections import deque
from contextlib import ExitStack

import ml_dtypes
import numpy as np

import concourse.bacc as bacc
import concourse.bass as bass
import concourse.mybir as mybir
import concourse.tile as tile
from concourse.bass_utils import run_bass_kernel_spmd
from concourse.masks import make_identity

HIDDEN = 4096
NH = 32
NKV = 8
HD = 128
THETA = 10000.0
S = 2048
NCORES = 8

QH = NH // NCORES          # 4 q heads per core
DQ = QH * HD               # 512 (per-core q/attn width)
DOUT = DQ + 2 * HD         # 768 = q heads + k + v projection width
MT = DOUT // 128           # 6 projection m-tiles (0..3 q, 4 k, 5 v)
KT = HIDDEN // 128         # 32 contraction tiles
KG = 8                     # x-load group: k-tiles per DMA (1MB bf16)
TCH = 512                  # token chunk (matmul moving dim)
NTCH = S // TCH            # 4
KVT = S // 128             # 16 kv tiles
SCALE = 1.0 / float(np.sqrt(HD))

F32 = mybir.dt.float32
BF16 = mybir.dt.bfloat16
EXP = mybir.ActivationFunctionType.Exp


def _rope(nc, pool, src, dst, cs, sn):
    """dst = src*cos + rotate_half(src)*sin, in [d, tok] layout.

    src is [128, n] (PSUM f32); dst is [128, n] bf16; cs/sn are [64, n]
    (the two 64-row halves share frequencies). rotate_half: rows 0:64 get
    -src[64:128], rows 64:128 get src[0:64]. NOTE: a tensor_tensor op may
    not take two SBUF inputs at different base partitions (walrus
    NCC_IBIR297), so the cross-half products go through [64]-row tiles at
    base partition 0 with the PSUM operand carrying the partition offset.
    """
    top, bot = src[0:64, :], src[64:128, :]
    ta = pool.tile([64, TCH], BF16, name="rope_a")
    tb = pool.tile([64, TCH], BF16, name="rope_b")
    nc.vector.tensor_mul(ta, top, cs)
    nc.vector.tensor_mul(tb, bot, sn)
    nc.vector.tensor_sub(dst[0:64, :], ta, tb)
    nc.vector.tensor_mul(ta, bot, cs)
    nc.vector.tensor_mul(tb, top, sn)
    nc.vector.tensor_add(dst[64:128, :], ta, tb)


def build_kernel_body(ctx: ExitStack, tc: tile.TileContext, outs, ins):
    nc = tc.nc
    xT, wqkv, ow, cos_t, sin_t, stair = (
        ins["xT"], ins["wqkv"], ins["ow"], ins["cos_t"], ins["sin_t"], ins["stair"],
    )
    out = outs["out"]

    # per-head bounce + gather buffers so each head's AllGather can fire as
    # soon as that head's attention is done (overlaps comm with compute).
    # Head 3 (the last one computed) gathers per q-chunk instead, so its
    # final AllGather only covers 512 tokens and o_proj isn't gated on a
    # full-head collective at the tail.
    attn_loc = [nc.dram_tensor(f"attn_loc{h}", [HD, S], BF16).ap()
                for h in range(QH - 1)]
    attn_gat = [nc.dram_tensor(f"attn_gat{h}", [NCORES * HD, S], BF16,
                               addr_space="Shared").ap()
                for h in range(QH - 1)]
    attn_loc3 = [nc.dram_tensor(f"attn_loc3_{c}", [HD, TCH], BF16).ap()
                 for c in range(NTCH)]
    attn_gat3 = [nc.dram_tensor(f"attn_gat3_{c}", [NCORES * HD, TCH], BF16,
                                addr_space="Shared").ap()
                 for c in range(NTCH)]

    singles = ctx.enter_context(tc.tile_pool(name="singles", bufs=1))
    stair_sb = singles.tile([128, 896], BF16)
    ones_sb = singles.tile([128, 128], BF16)

    # persistent projection outputs, [d, tok] layout
    qT = singles.tile([128, QH, S], BF16)    # q head h -> qT[:, h, :]
    kT = singles.tile([128, S], BF16)
    V = singles.tile([128, KVT, HD], BF16)   # V[:, j, :] = [tok 128, d 128]

    # ---- phase 1: QKV projection + RoPE --------------------------------
    with (
        tc.tile_pool(name="wq", bufs=1) as wp,
        tc.tile_pool(name="xt", bufs=3) as xp,
        tc.tile_pool(name="rope", bufs=2) as rp,
        tc.tile_pool(name="p1ps", bufs=1, space="PSUM") as pp1,
    ):
        cos_sb = wp.tile([64, S], F32)
        sin_sb = wp.tile([64, S], F32)
        vT = wp.tile([128, S], BF16)
        ident_sb = wp.tile([128, 128], BF16)
        nc.vector.memset(ones_sb, 1.0)
        make_identity(nc, ident_sb)

        x3 = xT.rearrange("(k p) s -> p k s", p=128)
        wq4 = wqkv.rearrange("(g k p) d -> p g k d", p=128, k=KG)
        # DMA order: the first two k-tiles of weights+x land first so the
        # first matmul starts ~2us in; after that, weight groups interleave
        # just-in-time with chunk-0 x groups so x loads aren't queued behind
        # the whole 6.3MB weight stream.
        w00 = wp.tile([128, 2, DOUT], BF16, name="w00", tag="w00")
        w01 = wp.tile([128, KG - 2, DOUT], BF16, name="w01", tag="w01")
        w_sb = [None] + [wp.tile([128, KG, DOUT], BF16, name=f"w{g}",
                                 tag=f"w{g}")
                         for g in range(1, KT // KG)]
        x00 = xp.tile([128, 2, TCH], BF16, name="x00", tag="x00")
        x01 = xp.tile([128, KG - 2, TCH], BF16, name="x01", tag="x01")
        nc.sync.dma_start(out=w00, in_=wq4[:, 0, 0:2, :])
        nc.sync.dma_start(out=x00, in_=x3[:, 0:2, 0:TCH])
        nc.sync.dma_start(out=w01, in_=wq4[:, 0, 2:KG, :])
        nc.sync.dma_start(out=x01, in_=x3[:, 2:KG, 0:TCH])

        def wslice(kg, ki, m):
            if kg == 0:
                wt, i = (w00, ki) if ki < 2 else (w01, ki - 2)
            else:
                wt, i = w_sb[kg], ki
            return wt[:, i, m * 128:(m + 1) * 128]

        xg_t0 = [None] * (KT // KG)
        for g in range(1, KT // KG):
            nc.sync.dma_start(out=w_sb[g], in_=wq4[:, g, :, :])
            xg = xp.tile([128, KG, TCH], BF16, name="xg")
            nc.sync.dma_start(
                out=xg, in_=x3[:, g * KG:(g + 1) * KG, 0:TCH])
            xg_t0[g] = xg
        nc.sync.dma_start(out=cos_sb, in_=cos_t)
        nc.sync.dma_start(out=sin_sb, in_=sin_t)
        nc.sync.dma_start(out=stair_sb, in_=stair)

        def chunk_epilogue_v(t):
            # V = vT.T for this chunk's kv tiles (PE transpose [d,tok]->[tok,d])
            for j in range(4 * t, 4 * t + 4):
                pv = pp1.tile([128, 128], BF16, name="pvt", tag="pvt")
                nc.tensor.transpose(pv, vT[:, j * 128:(j + 1) * 128], ident_sb)
                nc.scalar.copy(out=V[:, j, :], in_=pv)

        for t in range(NTCH - 1):
            ps = [pp1.tile([128, TCH], F32, name=f"p1_{m}", tag=f"p1_{m}")
                  for m in range(MT)]
            for kg in range(KT // KG):
                if t == 0:
                    xg = xg_t0[kg]  # None for kg==0: wslice/x00/x01 cover it
                else:
                    xg = xp.tile([128, KG, TCH], BF16, name="xg")
                    nc.sync.dma_start(
                        out=xg,
                        in_=x3[:, kg * KG:(kg + 1) * KG, t * TCH:(t + 1) * TCH])
                for ki in range(KG):
                    k = kg * KG + ki
                    if t == 0 and kg == 0:
                        xs = x00[:, ki, :] if ki < 2 else x01[:, ki - 2, :]
                    else:
                        xs = xg[:, ki, :]
                    for m in range(MT):
                        nc.tensor.matmul(
                            ps[m], lhsT=wslice(kg, ki, m), rhs=xs,
                            start=(k == 0), stop=(k == KT - 1),
                        )
            cs = cos_sb[:, t * TCH:(t + 1) * TCH]
            sn = sin_sb[:, t * TCH:(t + 1) * TCH]
            # k first: attention's score matmuls depend on kT
            _rope(nc, rp, ps[QH], kT[:, t * TCH:(t + 1) * TCH], cs, sn)
            nc.scalar.copy(out=vT[:, t * TCH:(t + 1) * TCH], in_=ps[QH + 1])
            for h in range(QH):
                _rope(nc, rp, ps[h], qT[:, h, t * TCH:(t + 1) * TCH], cs, sn)
            chunk_epilogue_v(t)

        # Last chunk runs m-outer / k-inner from a fully-prefetched x slab:
        # each projection finishes 32 matmuls apart, so its RoPE overlaps
        # the remaining matmul stream instead of draining after phase 1 —
        # attention starts ~immediately at the seam.
        t = NTCH - 1
        xbig = wp.tile([128, KT, TCH], BF16, name="xbig", tag="xbig")
        for g in range(KT // KG):
            nc.sync.dma_start(
                out=xbig[:, g * KG:(g + 1) * KG, :],
                in_=x3[:, g * KG:(g + 1) * KG, t * TCH:(t + 1) * TCH])
        ps = [pp1.tile([128, TCH], F32, name=f"p1_{m}", tag=f"p1_{m}")
              for m in range(MT)]
        cs = cos_sb[:, t * TCH:(t + 1) * TCH]
        sn = sin_sb[:, t * TCH:(t + 1) * TCH]
        for m in (QH, 0, 1, 2, 3, QH + 1):
            for k in range(KT):
                nc.tensor.matmul(
                    ps[m], lhsT=wslice(k // KG, k % KG, m),
                    rhs=xbig[:, k, :], start=(k == 0), stop=(k == KT - 1),
                )
            if m == QH:
                _rope(nc, rp, ps[m], kT[:, t * TCH:(t + 1) * TCH], cs, sn)
            elif m < QH:
                _rope(nc, rp, ps[m], qT[:, m, t * TCH:(t + 1) * TCH], cs, sn)
            else:
                nc.scalar.copy(out=vT[:, t * TCH:(t + 1) * TCH], in_=ps[m])
        chunk_epilogue_v(t)

    # ---- phase 2: attention heads with per-head AllGather ---------------
    ow3 = ow.rearrange("(k p) d -> p k d", p=128)
    out3 = out.rearrange("(b p) d -> p b d", p=128)
    ag3 = [attn_gat[h].rearrange("(r p) s -> p r s", p=128)
           for h in range(QH - 1)]
    ag3c = [attn_gat3[c].rearrange("(r p) s -> p r s", p=128)
            for c in range(NTCH)]
    owp = ctx.enter_context(tc.tile_pool(name="ow", bufs=32))

    with (
        tc.tile_pool(name="pt", bufs=4) as ptp,
        tc.tile_pool(name="ao", bufs=2) as aop,
        tc.tile_pool(name="p2sc", bufs=2, space="PSUM") as pp2,
        tc.tile_pool(name="p2acc", bufs=2, space="PSUM") as pa2,
    ):
        # One continuous rolling pipeline across ALL (head, q-chunk) units:
        # the PV/sum matmuls trail the score+exp emission by DEPTH kv-pairs
        # globally, so the PE never drains at chunk or head boundaries (which
        # previously cost ~2us each plus a HAM re-throttle to 1.2GHz).
        def allgather(ins_ap, outs_ap):
            nc.gpsimd.collective_compute(
                "AllGather",
                mybir.AluOpType.bypass,
                ins=[ins_ap],
                outs=[outs_ap],
                replica_groups=[list(range(NCORES))],
            )

        ows = [[None] * NCORES for _ in range(QH)]
        state = {}
        pending = deque()
        DEPTH = 2

        def emit_scores(u, p):
            h, c = u
            st = state[u]
            sc = pp2.tile([128, 2, TCH], F32, name="sc", tag="sc")
            for i in range(2):
                j = 2 * p + i
                nc.tensor.matmul(
                    sc[:, i, :], lhsT=kT[:, j * 128:(j + 1) * 128],
                    rhs=st["q"], start=True, stop=True)
            pt = ptp.tile([128, 2, TCH], BF16, name="pt", tag="pt")
            nc.scalar.activation(pt, sc, EXP, scale=SCALE)
            for i in range(2):
                j = 2 * p + i
                rdiag = j - 4 * c
                if rdiag >= 0:  # tile touches the causal diagonal
                    off = 384 - rdiag * 128
                    nc.vector.tensor_mul(
                        pt[:, i, :], pt[:, i, :], stair_sb[:, off:off + TCH])
            st["pts"][p] = pt

        def epilogue(u):
            h, c = u
            st = state.pop(u)
            rec = aop.tile([128, TCH], F32, name="rec")
            nc.vector.reciprocal(rec, st["ps"])
            ao = aop.tile([128, TCH], BF16, name="ao")
            nc.vector.tensor_mul(ao, st["po"], rec)
            if h == QH - 1:
                nc.sync.dma_start(out=attn_loc3[c][:, :], in_=ao)
                allgather(attn_loc3[c][:, :], attn_gat3[c][:, :])
            else:
                nc.sync.dma_start(
                    out=attn_loc[h][:, c * TCH:(c + 1) * TCH], in_=ao)
                if c == NTCH - 1:
                    allgather(attn_loc[h][:, :], attn_gat[h][:, :])
                    if h == 0:
                        # o_proj weights DMA (4.2MB bf16) rides under attention
                        for hh in range(QH):
                            for r in range(NCORES):
                                owk = owp.tile([128, DQ], BF16, name="owk",
                                               tag="owk")
                                nc.sync.dma_start(
                                    out=owk, in_=ow3[:, r * QH + hh, :])
                                ows[hh][r] = owk

        def emit_pv(u, p):
            st = state[u]
            pt = st["pts"].pop(p)
            jmax = st["jmax"]
            for i in range(2):
                j = 2 * p + i
                nc.tensor.matmul(st["po"], lhsT=V[:, j, :], rhs=pt[:, i, :],
                                 start=(j == 0), stop=(j == jmax))
                nc.tensor.matmul(st["ps"], lhsT=ones_sb, rhs=pt[:, i, :],
                                 start=(j == 0), stop=(j == jmax))
            st["done"] += 1
            if st["done"] == st["npair"]:
                epilogue(u)

        for u in [(h, c) for h in range(QH) for c in range(NTCH)]:
            h, c = u
            jmax = 4 * c + 3
            state[u] = dict(
                q=qT[:, h, c * TCH:(c + 1) * TCH], jmax=jmax,
                npair=(jmax + 1) // 2, done=0, pts={},
                po=pa2.tile([128, TCH], F32, name="po", tag="po"),
                ps=pa2.tile([128, TCH], F32, name="ps", tag="ps"))
            for p in range(state[u]["npair"]):
                emit_scores(u, p)
                pending.append((u, p))
                if len(pending) > DEPTH:
                    emit_pv(*pending.popleft())
        while pending:
            emit_pv(*pending.popleft())

    # ---- phase 3: o_proj, accumulated over all (head, rank) in PSUM -----
    with (
        tc.tile_pool(name="at", bufs=12) as atp,
        tc.tile_pool(name="stg", bufs=2) as stp,
        tc.tile_pool(name="p3ps", bufs=2, space="PSUM") as pp3,
    ):
        for g in range(S // TCH):
            ats = []
            for h in range(QH):
                at = atp.tile([128, NCORES, TCH], BF16, name="at", tag="at")
                if h == QH - 1:
                    nc.sync.dma_start(out=at, in_=ag3c[g][:, :, :])
                else:
                    nc.sync.dma_start(
                        out=at, in_=ag3[h][:, :, g * TCH:(g + 1) * TCH])
                ats.append(at)
            pc = pp3.tile([128, 4, TCH], F32, name="pc", tag="pc")
            for h in range(QH):
                for r in range(NCORES):
                    for mi in range(4):
                        nc.tensor.matmul(
                            pc[:, mi, :],
                            lhsT=ats[h][:, r, mi * 128:(mi + 1) * 128],
                            rhs=ows[h][r],
                            start=(h == 0 and r == 0),
                            stop=(h == QH - 1 and r == NCORES - 1),
                        )
            stg = stp.tile([128, 4, TCH], F32, name="stg")
            nc.scalar.copy(out=stg, in_=pc)
            nc.sync.dma_start(out=out3[:, 4 * g:4 * g + 4, :], in_=stg)


_NC_CACHE = None


def build_program():
    global _NC_CACHE
    if _NC_CACHE is not None:
        return _NC_CACHE
    nc = bacc.Bacc("TRN2", target_bir_lowering=False, debug=False,
                   num_devices=NCORES)
    ins = {
        "xT": nc.dram_tensor("xT", [HIDDEN, S], BF16, kind="ExternalInput").ap(),
        "wqkv": nc.dram_tensor("wqkv", [HIDDEN, DOUT], BF16,
                               kind="ExternalInput").ap(),
        "ow": nc.dram_tensor("ow", [HIDDEN, DQ], BF16, kind="ExternalInput").ap(),
        "cos_t": nc.dram_tensor("cos_t", [64, S], F32, kind="ExternalInput").ap(),
        "sin_t": nc.dram_tensor("sin_t", [64, S], F32, kind="ExternalInput").ap(),
        "stair": nc.dram_tensor("stair", [128, 896], BF16,
                                kind="ExternalInput").ap(),
    }
    outs = {"out": nc.dram_tensor("out", [S, DQ], F32, kind="ExternalOutput").ap()}
    with tile.TileContext(nc) as tc:
        with ExitStack() as ctx:
            build_kernel_body(ctx, tc, outs, ins)
    nc.compile()
    _NC_CACHE = nc
    return nc


def make_in_maps(hidden_states, position_ids, q_w, k_w, v_w, o_w):
    bf16 = ml_dtypes.bfloat16
    x = np.asarray(hidden_states, dtype=np.float32).reshape(S, HIDDEN)
    xT = np.ascontiguousarray(x.T).astype(bf16)
    pos = np.asarray(position_ids).reshape(S).astype(np.float64)
    inv = 1.0 / (THETA ** (np.arange(0, HD, 2, dtype=np.float64) / HD))
    fr = inv[:, None] * pos[None, :]                       # [64, S]
    cos_t = np.cos(fr).astype(np.float32)
    sin_t = np.sin(fr).astype(np.float32)
    u = np.arange(896, dtype=np.int64)[None, :]
    kvi = np.arange(128, dtype=np.int64)[:, None]
    stair = ((u - kvi) >= 384).astype(bf16)                # [128, 896]

    q_w = np.asarray(q_w, dtype=np.float32)
    k_w = np.asarray(k_w, dtype=np.float32)
    v_w = np.asarray(v_w, dtype=np.float32)
    o_w = np.asarray(o_w, dtype=np.float32)

    in_maps = []
    for c in range(NCORES):
        wqkv = np.ascontiguousarray(np.concatenate(
            [q_w[:, c * DQ:(c + 1) * DQ],
             k_w[:, c * HD:(c + 1) * HD],
             v_w[:, c * HD:(c + 1) * HD]], axis=1)).astype(bf16)
        owc = np.ascontiguousarray(o_w[:, c * DQ:(c + 1) * DQ]).astype(bf16)
        in_maps.append({"xT": xT, "wqkv": wqkv, "ow": owc,
                        "cos_t": cos_t, "sin_t": sin_t, "stair": stair})
    return in_maps


def run(inputs: dict, trace: bool = False):
    """Run on the 8 NeuronCores; returns (full_output, BassKernelResults)."""
    nc = build_program()
    in_maps = make_in_maps(**inputs)
    res = run_bass_kernel_spmd(nc, in_maps, core_ids=list(range(NCORES)),
                               trace=trace)
    full = np.concatenate([res.results[c]["out"] for c in range(NCORES)], axis=1)
    return full.reshape(1, S, HIDDEN), res


def kernel(**inputs) -> np.ndarray:
    out, _ = run(inputs)
    return out
